# revision 6
# baseline (speedup 1.0000x reference)
"""Trainium2 Bass kernel for nn_HandCodedAttentionLayer (B=4,S=2048,E=1024,H=16).

Sharding: 8 cores = (batch b in 0..3) x (head-group hg in 0..1); each core owns
one batch element and 8 of the 16 heads.  Per core:
  QT/KT = W{q,k}T_hg.T @ xT   (transposed projections, [512, 2048], bf16)
  V     = xT.T @ WvT_hg       (natural layout, [2048, 512], bf16, fused 1-cols)
  per head: scoresT = KT_h.T @ QT_h  -> exp on ACT -> oT = [V_h|1].T @ expT
            (row 64 of oT = softmax denominator; fused into the matmul)
  out_partial = O.T @ WoT_hg  (+ bo on the hg==0 core only, via zeroed input)
Host sums the two hg partials per batch.

x and Wq/Wk/Wv are passed from the host pre-transposed and pre-cast to bf16;
Wo stays fp32 and the final matmul reads it as float32r (fp22, single pass).
The device never transposes anything.
"""

import numpy as np
from contextlib import ExitStack

import ml_dtypes

import concourse.bacc as bacc
import concourse.bass as bass
import concourse.mybir as mybir
import concourse.tile as tile
from concourse import bass_utils

B, S, E, H = 4, 2048, 1024, 16
D = E // H              # 64
HL = H // 2             # 8 heads per core
DH = HL * D             # 512 local head dims
P = 128
NE = E // P             # 8 e-tiles
NST = S // P            # 16 s-tiles
SQB = 512               # sq block (psum free dim)
NSQB = S // SQB         # 4
NPAIR = DH // P         # 4 head pairs per core
INV_SCALE = 1.0 / float(np.sqrt(D))

F32 = mybir.dt.float32
F32R = mybir.dt.float32r
BF16 = mybir.dt.bfloat16

_cache = {}


def _r(ap):
    return ap.bitcast(F32R)


def _build():
    nc = bacc.Bacc("TRN2", target_bir_lowering=False, debug=False, num_devices=8)

    xT_d = nc.dram_tensor("xT", [E, S], BF16, kind="ExternalInput")
    wq_d = nc.dram_tensor("wqT", [E, DH], BF16, kind="ExternalInput")
    wk_d = nc.dram_tensor("wkT", [E, DH], BF16, kind="ExternalInput")
    wv_d = nc.dram_tensor("wvT", [E, DH], BF16, kind="ExternalInput")
    wo_d = nc.dram_tensor("woT", [DH, E], F32R, kind="ExternalInput")
    bq_d = nc.dram_tensor("bq", [DH], F32, kind="ExternalInput")
    bk_d = nc.dram_tensor("bk", [DH], F32, kind="ExternalInput")
    bv_d = nc.dram_tensor("bv", [DH], F32, kind="ExternalInput")
    bo_d = nc.dram_tensor("bo", [E], F32, kind="ExternalInput")
    out_d = nc.dram_tensor("out", [S, E], F32, kind="ExternalOutput")

    with tile.TileContext(nc) as tc, ExitStack() as ctx:
        qkv = ctx.enter_context(tc.tile_pool(name="qkv", bufs=1))
        smalls = ctx.enter_context(tc.tile_pool(name="smalls", bufs=2))
        ps_sc = ctx.enter_context(tc.tile_pool(name="ps_sc", bufs=2, space="PSUM"))
        ps_ot = ctx.enter_context(tc.tile_pool(name="ps_ot", bufs=2, space="PSUM"))
        ps_pj = ctx.enter_context(tc.tile_pool(name="ps_pj", bufs=2, space="PSUM"))

        # persistent intermediates (live through attention + final)
        qt = qkv.tile([P, NPAIR, S], BF16, tag="qt")
        kt = qkv.tile([P, NPAIR, S], BF16, tag="kt")
        vaug = qkv.tile([P, NST, HL * (D + 1)], BF16, tag="vaug")  # [., ., 520]
        o_sb = qkv.tile([P, NPAIR, S], F32R, tag="o")
        vaug4 = vaug[:].rearrange("p t (h x) -> p t h x", h=HL)
        nc.vector.memset(vaug4[:, :, :, D : D + 1], 1.0)

        # ---- phase 1: projections (xT + W in a scoped pool, freed after) ----
        with tc.tile_pool(name="loads", bufs=1) as loads:
            xT = loads.tile([P, NE, S], BF16, tag="xT")
            xT_src = xT_d.ap().rearrange("(o p) s -> p o s", p=P)
            for e in range(NE):
                nc.sync.dma_start(xT[:, e, :], xT_src[:, e, :])

            wq = loads.tile([P, NE, DH], BF16, tag="wq")
            wk = loads.tile([P, NE, DH], BF16, tag="wk")
            wv = loads.tile([P, NE, DH], BF16, tag="wv")
            for wsb, wd in ((wq, wq_d), (wk, wk_d), (wv, wv_d)):
                nc.sync.dma_start(wsb[:], wd.ap().rearrange("(o p) m -> p o m", p=P))

            bqs = loads.tile([P, NPAIR], F32, tag="bq")
            bks = loads.tile([P, NPAIR], F32, tag="bk")
            for bsb, bd in ((bqs, bq_d), (bks, bk_d)):
                nc.sync.dma_start(bsb[:], bd.ap().rearrange("(o p) -> p o", p=P))
            bv_row = loads.tile([1, DH], F32, tag="bvr")
            nc.sync.dma_start(bv_row[:], bv_d.ap().rearrange("(o m) -> o m", o=1))
            bvb = loads.tile([P, DH], F32, tag="bvb")
            nc.gpsimd.partition_broadcast(bvb[:], bv_row[:])

            for pair in range(NPAIR):
                for sqb in range(NSQB):
                    for wsb, bsb, dst in ((wq, bqs, qt), (wk, bks, kt)):
                        ps = ps_pj.tile([P, SQB], F32, tag="pj")
                        for e in range(NE):
                            nc.tensor.matmul(
                                ps[:],
                                wsb[:, e, pair * P : (pair + 1) * P],
                                xT[:, e, sqb * SQB : (sqb + 1) * SQB],
                                start=(e == 0),
                                stop=(e == NE - 1),
                            )
                        nc.vector.tensor_scalar_add(
                            dst[:, pair, sqb * SQB : (sqb + 1) * SQB],
                            ps[:],
                            bsb[:, pair : pair + 1],
                        )

            for st in range(NST):
                ps = ps_pj.tile([P, SQB], F32, tag="pj")
                for e in range(NE):
                    nc.tensor.matmul(
                        ps[:],
                        xT[:, e, st * P : (st + 1) * P],
                        wv[:, e, :],
                        start=(e == 0),
                        stop=(e == NE - 1),
                    )
                nc.vector.tensor_tensor(
                    vaug4[:, st, :, 0:D],
                    ps[:].rearrange("p (h d) -> p h d", h=HL),
                    bvb[:].rearrange("p (h d) -> p h d", h=HL),
                    mybir.AluOpType.add,
                )

        # ---- phases 2+3: attention + final projection ----
        with (
            tc.tile_pool(name="expp", bufs=2) as expp,
            tc.tile_pool(name="wop", bufs=1) as wop,
            tc.tile_pool(name="outp", bufs=3) as outp,
        ):
            wo = wop.tile([P, NPAIR, E], F32R, tag="wo")
            nc.sync.dma_start(wo[:], wo_d.ap().rearrange("(o p) m -> p o m", p=P))
            bo_row = wop.tile([1, E], F32, tag="bor")
            nc.sync.dma_start(bo_row[:], bo_d.ap().rearrange("(o m) -> o m", o=1))
            bob = wop.tile([P, E], F32, tag="bob")
            nc.gpsimd.partition_broadcast(bob[:], bo_row[:])

            def scores_exp(pair, hh, sqb):
                """scoresT + exp for head pair*2+hh, query block sqb."""
                pb = hh * D
                expT = expp.tile([P, NST, SQB], BF16, tag="expT")
                for c in range(NST // 2):
                    ps = ps_sc.tile([P, 2, SQB], F32, tag="sc")
                    for j in range(2):
                        skt = c * 2 + j
                        nc.tensor.matmul(
                            ps[:, j, :],
                            kt[pb : pb + D, pair, skt * P : (skt + 1) * P],
                            qt[pb : pb + D, pair, sqb * SQB : (sqb + 1) * SQB],
                            start=True,
                            stop=True,
                        )
                    nc.scalar.activation(
                        expT[:, c * 2 : c * 2 + 2, :],
                        ps[:],
                        mybir.ActivationFunctionType.Exp,
                        scale=INV_SCALE,
                    )
                return expT

            def ot_norm(pair, hh, sqb, expT):
                """oT = [V_h|1].T @ expT; normalize by row D; write O."""
                h = pair * 2 + hh
                pb = hh * D
                ps = ps_ot.tile([P, SQB], F32, tag="ot")
                for skt in range(NST):
                    nc.tensor.matmul(
                        ps[: D + 1, :],
                        vaug[:, skt, h * (D + 1) : (h + 1) * (D + 1)],
                        expT[:, skt, :],
                        start=(skt == 0),
                        stop=(skt == NST - 1),
                    )
                recip = smalls.tile([1, SQB], F32, tag="recip")
                nc.vector.reciprocal(recip[:], ps[D : D + 1, :])
                rb = smalls.tile([D, SQB], F32, tag="rb")
                nc.gpsimd.partition_broadcast(rb[:], recip[:])
                nc.vector.tensor_tensor(
                    o_sb[pb : pb + D, pair, sqb * SQB : (sqb + 1) * SQB],
                    ps[0:D, :],
                    rb[:],
                    mybir.AluOpType.mult,
                )

            # attention, software-pipelined one block ahead
            prev = None
            for pair in range(NPAIR):
                for hh in range(2):
                    for sqb in range(NSQB):
                        expT = scores_exp(pair, hh, sqb)
                        if prev is not None:
                            ot_norm(*prev)
                        prev = (pair, hh, sqb, expT)
            ot_norm(*prev)

            # final projection out[s, e] = O.T @ WoT + bo
            for st in range(NST):
                for eb in range(E // SQB):
                    ps = ps_pj.tile([P, SQB], F32, tag="pj")
                    for k in range(NPAIR):
                        nc.tensor.matmul(
                            ps[:],
                            o_sb[:, k, st * P : (st + 1) * P],
                            wo[:, k, eb * SQB : (eb + 1) * SQB],
                            start=(k == 0),
                            stop=(k == NPAIR - 1),
                        )
                    ot = outp.tile([P, SQB], F32, tag="out")
                    nc.vector.tensor_add(
                        ot[:], ps[:], bob[:, eb * SQB : (eb + 1) * SQB]
                    )
                    nc.sync.dma_start(
                        out_d.ap()[st * P : (st + 1) * P, eb * SQB : (eb + 1) * SQB],
                        ot[:],
                    )

    nc.compile()
    return nc


class _Runner:
    """Persistent jitted shard_map executor for a Bass program (axon/PJRT).

    Mirrors concourse.bass2jax.run_bass_via_pjrt but caches the jitted
    callable so repeat kernel() calls don't re-trace/re-compile, and exposes
    device-resident execution for timing.
    """

    def __init__(self, nc, n_cores=8):
        import jax
        from jax.sharding import Mesh, PartitionSpec
        from jax.experimental.shard_map import shard_map
        import concourse.bass2jax as b2j
        import concourse.mybir as _mybir

        b2j.install_neuronx_cc_hook()
        assert nc.dbg_addr is None
        partition_name = (
            nc.partition_id_tensor.name if nc.partition_id_tensor else None
        )

        self.jax = jax
        self.n_cores = n_cores
        in_names, out_names, out_avals = [], [], []
        for alloc in nc.m.functions[0].allocations:
            if not isinstance(alloc, _mybir.MemoryLocationSet):
                continue
            name = alloc.memorylocations[0].name
            if alloc.kind == "ExternalInput":
                if name != partition_name:
                    in_names.append(name)
            elif alloc.kind == "ExternalOutput":
                out_names.append(name)
                out_avals.append(
                    jax.core.ShapedArray(
                        tuple(alloc.tensor_shape), _mybir.dt.np(alloc.dtype)
                    )
                )
        self.in_names, self.out_names, self.out_avals = in_names, out_names, out_avals
        n_params, n_outs = len(in_names), len(out_avals)
        all_names = tuple(in_names + out_names)
        if partition_name is not None:
            all_names = all_names + (partition_name,)

        def _body(*args):
            operands = list(args)
            if partition_name is not None:
                operands.append(b2j.partition_id_tensor())
            outs = b2j._bass_exec_p.bind(
                *operands,
                out_avals=tuple(out_avals),
                in_names=all_names,
                out_names=tuple(out_names),
                lowering_input_output_aliases=(),
                sim_require_finite=True,
                sim_require_nnan=True,
                nc=nc,
            )
            return tuple(outs)

        devices = jax.devices()[:n_cores]
        self.mesh = Mesh(np.asarray(devices), ("core",))
        self.pspec = PartitionSpec("core")
        in_specs = (self.pspec,) * (n_params + n_outs)
        out_specs = (self.pspec,) * n_outs
        self.fn = jax.jit(
            shard_map(_body, mesh=self.mesh, in_specs=in_specs,
                      out_specs=out_specs, check_rep=False),
            donate_argnums=tuple(range(n_params, n_params + n_outs)),
            keep_unused=True,
        )

    def device_inputs(self, in_maps):
        """Concat per-core input dicts and place on devices."""
        import jax
        from jax.sharding import NamedSharding
        sh = NamedSharding(self.mesh, self.pspec)
        concat = [
            np.concatenate([np.asarray(m[name]) for m in in_maps], axis=0)
            for name in self.in_names
        ]
        return [jax.device_put(a, sh) for a in concat]

    def zeros(self):
        import jax.numpy as jnp
        from jax.sharding import NamedSharding
        sh = NamedSharding(self.mesh, self.pspec)
        return [
            jnp.zeros((self.n_cores * av.shape[0], *av.shape[1:]), av.dtype,
                      device=sh)
            for av in self.out_avals
        ]

    def run(self, dev_in):
        outs = self.fn(*dev_in, *self.zeros())
        self.jax.block_until_ready(outs)
        return outs

    def run_numpy(self, in_maps):
        outs = self.run(self.device_inputs(in_maps))
        return [
            {
                name: np.asarray(outs[i]).reshape(
                    self.n_cores, *self.out_avals[i].shape
                )[c]
                for i, name in enumerate(self.out_names)
            }
            for c in range(self.n_cores)
        ]


def get_runner():
    if "runner" not in _cache:
        _cache["runner"] = _Runner(_get_nc())
    return _cache["runner"]


def _get_nc():
    if "nc" not in _cache:
        _cache["nc"] = _build()
    return _cache["nc"]


def make_in_maps(x, Wq, bq, Wk, bk, Wv, bv, Wo, bo):
    def b16(a):
        return np.ascontiguousarray(a).astype(ml_dtypes.bfloat16)

    zeros_bo = np.zeros_like(bo)
    in_maps = []
    for core in range(8):
        b, hg = core // 2, core % 2
        sl = slice(hg * DH, (hg + 1) * DH)
        in_maps.append(
            {
                "xT": b16(x[b].T),
                "wqT": b16(Wq[sl, :].T),
                "wkT": b16(Wk[sl, :].T),
                "wvT": b16(Wv[sl, :].T),
                "woT": np.ascontiguousarray(Wo[:, sl].T),
                "bq": np.ascontiguousarray(bq[sl]),
                "bk": np.ascontiguousarray(bk[sl]),
                "bv": np.ascontiguousarray(bv[sl]),
                "bo": bo if hg == 0 else zeros_bo,
            }
        )
    return in_maps


def kernel(x, Wq, bq, Wk, bk, Wv, bv, Wo, bo):
    x, Wq, bq, Wk, bk, Wv, bv, Wo, bo = (
        np.asarray(a, dtype=np.float32)
        for a in (x, Wq, bq, Wk, bk, Wv, bv, Wo, bo)
    )
    runner = get_runner()
    res = runner.run_numpy(make_in_maps(x, Wq, bq, Wk, bk, Wv, bv, Wo, bo))

    out = np.zeros((B, S, E), dtype=np.float32)
    for b in range(B):
        out[b] = res[2 * b]["out"] + res[2 * b + 1]["out"]
    return out
